# revision 4
# baseline (speedup 1.0000x reference)
"""Raw-Bacc (no TileContext) CenterLoss kernel.

Per core (128 batch rows):
  SP queue:  DMA labels [128,1] int32 -> SBUF
  ACT queue: DMA x [128,512] -> SBUF
  Pool:      indirect DMA gathers centers[labels] rows into SBUF
  DVE:       diff = x - c, square, row-reduce, clip to [1e-12, 1e12]
  PE:        ones-matmul reduces the 128 partition values to one scalar
  DVE:       PSUM -> SBUF copy;  SP: DMA scalar out
Host sums the 8 per-core partials (the all-reduce) and divides by B.
"""

import numpy as np

_BATCH = 1024
_FEAT = 512
_NCLASSES = 10000
_NCORES = 8
_ROWS = _BATCH // _NCORES  # 128
_P = 128

_state = {}


def _build_nc_raw(decoy=False):
    import concourse.bass as bass
    import concourse.mybir as mybir
    from concourse import bacc

    f32 = mybir.dt.float32
    i32 = mybir.dt.int32
    nc = bacc.Bacc("TRN2", target_bir_lowering=False, debug=False)
    x_d = nc.dram_tensor("x", [_ROWS, _FEAT], f32, kind="ExternalInput").ap()
    labels_d = nc.dram_tensor("labels", [_ROWS, 1], i32, kind="ExternalInput").ap()
    centers_d = nc.dram_tensor(
        "centers", [_NCLASSES, _FEAT], f32, kind="ExternalInput"
    ).ap()
    out_d = nc.dram_tensor("out", [1, 1], f32, kind="ExternalOutput").ap()

    with (
        nc.sbuf_tensor("labels_t", [_ROWS, 1], i32) as labels_t,
        nc.sbuf_tensor("decoy_t", [1, 4], f32) as decoy_t,
        nc.sbuf_tensor("x_t", [_P, _FEAT], f32) as x_t,
        nc.sbuf_tensor("c_t", [_P, _FEAT], f32) as c_t,
        nc.sbuf_tensor("diff_t", [_P, _FEAT], f32) as diff_t,
        nc.sbuf_tensor("sq_t", [_P, _FEAT], f32) as sq_t,
        nc.sbuf_tensor("d_t", [_P, 1], f32) as d_t,
        nc.sbuf_tensor("dc_t", [_P, 1], f32) as dc_t,
        nc.sbuf_tensor("ones_t", [_P, 1], f32) as ones_t,
        nc.sbuf_tensor("res_t", [1, 1], f32) as res_t,
        nc.psum_tensor("acc_t", [1, 1], f32) as acc_t,
        nc.semaphore("lab_sem") as lab_sem,
        nc.semaphore("decoy_sem") as decoy_sem,
        nc.semaphore("x_sem") as x_sem,
        nc.semaphore("c_sem") as c_sem,
        nc.semaphore("dve_sem") as dve_sem,
        nc.semaphore("m_sem") as m_sem,
        nc.semaphore("o_sem") as o_sem,
    ):
        # labels on the SP HWDGE queue so the gather can start ASAP
        nc.sync.dma_start(labels_t.ap(), labels_d).then_inc(lab_sem, 16)
        # x on the ACT HWDGE queue, overlaps the gather
        nc.scalar.dma_start(x_t.ap(), x_d).then_inc(x_sem, 16)
        nc.vector.memset(ones_t.ap(), 1.0)

        if decoy:
            # tiny SWDGE DMA warms the Pool dynamic-DMA path so the real
            # gather's ucode drain is cheap
            nc.gpsimd.dma_start(decoy_t.ap(), centers_d[0:1, 0:4]).then_inc(
                decoy_sem, 16
            )

        nc.gpsimd.wait_ge(lab_sem, 16)
        nc.gpsimd.indirect_dma_start(
            out=c_t.ap(),
            out_offset=None,
            in_=centers_d,
            in_offset=bass.IndirectOffsetOnAxis(ap=labels_t.ap()[:, :1], axis=0),
        ).then_inc(c_sem, 16)
        if decoy:
            nc.gpsimd.wait_ge(decoy_sem, 16)

        nc.vector.wait_ge(x_sem, 16)
        nc.vector.wait_ge(c_sem, 16)
        nc.vector.tensor_tensor(
            out=diff_t.ap(), in0=x_t.ap(), in1=c_t.ap(), op=mybir.AluOpType.subtract
        ).then_inc(dve_sem, 1)
        nc.vector.wait_ge(dve_sem, 1)
        nc.vector.tensor_tensor(
            out=sq_t.ap(), in0=diff_t.ap(), in1=diff_t.ap(), op=mybir.AluOpType.mult
        ).then_inc(dve_sem, 1)
        nc.vector.wait_ge(dve_sem, 2)
        nc.vector.reduce_sum(
            out=d_t.ap(), in_=sq_t.ap(), axis=mybir.AxisListType.X
        ).then_inc(dve_sem, 1)
        nc.vector.wait_ge(dve_sem, 3)
        nc.vector.tensor_scalar(
            out=dc_t.ap(),
            in0=d_t.ap(),
            scalar1=1e-12,
            scalar2=1e12,
            op0=mybir.AluOpType.max,
            op1=mybir.AluOpType.min,
        ).then_inc(dve_sem, 1)

        nc.tensor.wait_ge(dve_sem, 4)
        nc.tensor.matmul(
            acc_t.ap(), lhsT=dc_t.ap(), rhs=ones_t.ap(), start=True, stop=True
        ).then_inc(m_sem, 1)

        nc.vector.wait_ge(m_sem, 1)
        nc.vector.tensor_copy(out=res_t.ap(), in_=acc_t.ap()).then_inc(dve_sem, 1)

        nc.sync.wait_ge(dve_sem, 5)
        nc.sync.dma_start(out_d, res_t.ap()).then_inc(o_sem, 16)

    nc.compile()
    return nc


def _run(x, labels, centers, trace=False, decoy=False):
    from concourse.bass_utils import run_bass_kernel_spmd

    key = ("nc", decoy)
    if key not in _state:
        _state[key] = _build_nc_raw(decoy=decoy)
    nc = _state[key]

    x = np.ascontiguousarray(np.asarray(x, dtype=np.float32)).reshape(
        _NCORES, _ROWS, _FEAT
    )
    lab = (
        np.ascontiguousarray(np.asarray(labels))
        .astype(np.int32)
        .reshape(_NCORES, _ROWS, 1)
    )
    cen = np.ascontiguousarray(np.asarray(centers, dtype=np.float32))
    in_maps = [{"x": x[i], "labels": lab[i], "centers": cen} for i in range(_NCORES)]
    res = run_bass_kernel_spmd(nc, in_maps, core_ids=list(range(_NCORES)), trace=trace)
    total = 0.0
    for r in res.results:
        total += float(r["out"][0, 0])
    loss = total / _BATCH + (_NCLASSES - 1) * 1e-12
    return np.float32(loss), res


def kernel(x, labels, centers):
    loss, _ = _run(x, labels, centers, trace=False, decoy=True)
    return loss
